# revision 24
# baseline (speedup 1.0000x reference)
"""MoE expert-collection kernel for 8 Trainium2 NeuronCores.

Problem (hardcoded shapes):
  x          [8192, 1024] f32
  expert_idx [8192]       int    (values 0..7)
  Wr         [8, 1024, 1024] f32, br [8, 1024] f32   (routing experts)
  Ws         [2, 1024, 1024] f32, bs [2, 1024] f32   (shared experts)
  out[n] = silu(x[n] @ Wr[e_n] + br[e_n]) + sum_s silu(x[n] @ Ws[s] + bs[s])

Strategy (expert parallel, host-side all-to-all):
  - Host sorts tokens by expert. Core e computes, over the fixed sorted-order
    token window [e*1024, (e+1)*1024):
      * silu(x @ Ws[0] + bs[0]) + silu(x @ Ws[1] + bs[1])  (shared experts,
        data-parallel over tokens -> perfectly balanced)
      * silu(x @ Wr[e] + br[e])  (routing expert e; the window approximates
        expert e's token range, so almost every routed token is covered)
  - Host combines: out = concat(shared windows); window tokens of expert e
    get the routed rows added from core e's output. The <=1% of routed
    tokens that fall outside their expert's window (count skew) are computed
    on the host in f32 during the gather -- this keeps the device program a
    uniform SPMD shape with zero padding columns.
  - Matmuls run in bf16 (fp32 PSUM accumulation): 1 PE cycle/row vs 4 for f32.

Schedule (built from trace analysis):
  - Phase order: shared j=0 (Ws0) -> shared j=1 (Ws1) -> routed (Wr[e]).
  - Shared phases are k-OUTER over (m-group, 512-col chunk) tiles with all 8
    PSUM banks live (two sets of 4 m-tiles, alternating per phase so a
    bank's silu has a full ~7us phase to drain before reuse). The first
    phase's working set is only half a W k-tile + one xu k-tile chunk per
    k-step, so the PE starts on real data as soon as the first two ~128KB
    DMAs land (~10.5us, right after the fixed ~7us engine preamble).
  - A short 5-matmul warmup on a memset tile occupies the PE from ~7.3us to
    ~10.5us: it releases the HAM clock throttle (cold PE runs at 1.2GHz vs
    2.4GHz warm) ~3us earlier without delaying the first real matmul.
  - Weight loads go on the scalar HWDGE ring (m-group halves, interleaved
    per k to match phase consumption), activation loads + all stores on the
    sync ring. Stores only begin after all loads drained. xu is staged
    host-side in [P, KT, S] layout so every DMA is a contiguous >=1KB run
    per partition.
  - Outputs are stored as bf16 (host upcasts): halves store traffic so DMA
    never contends with the weight prefetch.
  - The routed phase is m-outer/k-inner (per (m,k) both column chunks share
    one weight load); the last m-tile splits its second chunk into 2x256 so
    the end-of-kernel silu->store drain is minimal.
"""

import contextlib
import ctypes
import sys
import types

import numpy as np
import ml_dtypes

import concourse.mybir as mybir
import concourse.tile as tile
from concourse import bacc
from concourse import bass_utils

N_CORES = 8
D = 1024          # d_in == d_out
P = 128           # partitions
KT = D // P       # 8 k-tiles
NJ = 3            # matrices per core: Ws[0], Ws[1], Wr[e]
N_EXPERTS = 8
S = 8192 // N_CORES  # tokens per core (1024)
CH = 512          # PSUM bank chunk (512 fp32)

BF16 = mybir.dt.bfloat16
F32 = mybir.dt.float32

# exposed for test.py introspection
last_results = None
last_nc = None
last_in_maps = None

_program_cache = {}


def _install_ntff_hook_fallback():
    """Some containers (including this one) lack antenv.axon_hooks, but
    concourse's run_bass_kernel_spmd imports it unconditionally when tracing
    is requested (BASS_TRACE=1). Provide it: a ctypes port of
    trn_boot._ntff_profile_via_ctypes driving NRT profiling through the axon
    PJRT plugin, or a None hook (= trace gracefully skipped) if unavailable."""
    if "antenv.axon_hooks" in sys.modules:
        return
    try:
        import antenv.axon_hooks  # noqa: F401
        return
    except ImportError:
        pass

    hook = None
    try:
        lib = ctypes.CDLL("/opt/axon/libaxon_pjrt.so")
        if hasattr(lib, "axon_start_nrt_profile"):
            lib.axon_start_nrt_profile.argtypes = [
                ctypes.POINTER(ctypes.c_int64),
                ctypes.c_size_t,
            ]
            lib.axon_start_nrt_profile.restype = ctypes.c_int64
            lib.axon_stop_nrt_profile.argtypes = [ctypes.c_char_p]
            lib.axon_stop_nrt_profile.restype = ctypes.c_int64

            @contextlib.contextmanager
            def _hook(output_dir, device_ids):
                import jax

                jax.devices()  # force PJRT init so the axon client exists
                if device_ids:
                    ids = (ctypes.c_int64 * len(device_ids))(*device_ids)
                    rc = lib.axon_start_nrt_profile(ids, len(device_ids))
                else:
                    rc = lib.axon_start_nrt_profile(None, 0)
                if rc != 0:
                    raise RuntimeError(f"axon_start_nrt_profile rc={rc}")
                try:
                    yield
                finally:
                    n = lib.axon_stop_nrt_profile(str(output_dir).encode())
                    if n < 0:
                        raise RuntimeError(f"axon_stop_nrt_profile rc={n}")

            hook = _hook
    except OSError:
        pass

    mod = types.ModuleType("antenv.axon_hooks")
    mod.get_axon_ntff_profile_hook = lambda: hook
    mod.set_axon_ntff_profile_hook = lambda h: None
    sys.modules["antenv.axon_hooks"] = mod


_install_ntff_hook_fallback()


def _build_program():
    nc = bacc.Bacc(
        "TRN2",
        target_bir_lowering=False,
        debug=False,
        enable_asserts=False,
        num_devices=N_CORES,
    )
    xu_d = nc.dram_tensor("xu", [P, KT, S], BF16, kind="ExternalInput")
    W_d = nc.dram_tensor("W", [NJ, D, D], BF16, kind="ExternalInput")
    b_d = nc.dram_tensor("b", [P, NJ * KT], F32, kind="ExternalInput")
    outr_d = nc.dram_tensor("outr", [D, S], BF16, kind="ExternalOutput")
    outs_d = nc.dram_tensor("outs", [D, S], BF16, kind="ExternalOutput")

    with tile.TileContext(nc) as tc:
        with (
            tc.tile_pool(name="const", bufs=1) as constp,
            tc.tile_pool(name="wpool", bufs=1) as wp,
            tc.tile_pool(name="xpool", bufs=1) as xp,
            tc.tile_pool(name="sil1p", bufs=1) as sil1p,
            tc.tile_pool(name="sil2p", bufs=3) as sil2p,
            tc.tile_pool(name="outp", bufs=4) as outp,
            tc.tile_pool(name="psum", bufs=1, space="PSUM") as psump,
        ):
            w_t = wp.tile([P, NJ, KT, D], BF16)
            xu_t = xp.tile([P, KT, S], BF16)
            bias_t = constp.tile([P, NJ * KT], F32)
            sil1_t = sil1p.tile([P, KT, S], BF16)

            # --- PE warmup: a memset tile + 5 dummy matmuls sized to end
            # right when the first real operands land (~10.5us), releasing
            # the HAM clock throttle early without delaying real work.
            warm_sb = constp.tile([P, 640], BF16, name="warm_sb")
            nc.vector.memset(warm_sb[:], 0.0)
            warm_ps = psump.tile([P, CH], F32, tag="ps0", name="warm_ps")
            for _ in range(4):
                nc.tensor.matmul(
                    warm_ps[:], warm_sb[:, :P], warm_sb[:, P : P + CH],
                    start=True, stop=True,
                )

            # --- input DMA: two parallel HWDGE issue streams, ordered by
            # first-use so the wires never carry "future" data during the
            # ramp. scalar ring: weights, split into m-group halves
            # (phase (mgA,c0) consumes h0 k-tiles, phase (mgB,c1) h1).
            for h in range(2):
                for k in range(KT):
                    nc.scalar.dma_start(
                        w_t[:, 0, k, h * CH : (h + 1) * CH],
                        W_d[0, k * P : (k + 1) * P, h * CH : (h + 1) * CH],
                    )
            nc.scalar.dma_start(
                w_t[:, 1, :, :], W_d[1].rearrange("(kt p) n -> p kt n", p=P)
            )
            nc.scalar.dma_start(
                w_t[:, 2, :, :], W_d[2].rearrange("(kt p) n -> p kt n", p=P)
            )
            # sync ring: activations, k-paced
            for k in range(KT):
                nc.sync.dma_start(xu_t[:, k, 0:CH], xu_d[:, k, 0:CH])
            nc.sync.dma_start(bias_t[:], b_d[:])
            for k in range(KT):
                nc.sync.dma_start(xu_t[:, k, CH:S], xu_d[:, k, CH:S])

            # --- shared experts, k-outer: phase = (j, m-group, col-chunk).
            # Per j the order (A,c0),(B,c0),(A,c1),(B,c1) alternates the PSUM
            # bank set every phase and needs at most 1MB of fresh input per
            # phase (phase 1: W h0 + xu c0 paced per-k; phase 2: W h1 only;
            # phase 3: xu c1 only; phase 4 fully cached).
            for j in range(2):
                for mg, (c0, c1) in ((0, (0, CH)), (1, (0, CH)),
                                     (0, (CH, S)), (1, (CH, S))):
                    ps = [
                        psump.tile(
                            [P, CH], F32, tag=f"ps{mg * 4 + i}",
                            name=f"pss_{j}_{mg}_{c0}_{i}",
                        )
                        for i in range(4)
                    ]
                    for k in range(KT):
                        for i in range(4):
                            m = mg * 4 + i
                            nc.tensor.matmul(
                                ps[i][:],
                                w_t[:, j, k, m * P : (m + 1) * P],
                                xu_t[:, k, c0:c1],
                                start=(k == 0),
                                stop=(k == KT - 1),
                            )
                    for i in range(4):
                        m = mg * 4 + i
                        bidx = j * KT + m
                        if j == 0:
                            nc.scalar.activation(
                                sil1_t[:, m, c0:c1],
                                ps[i][:],
                                mybir.ActivationFunctionType.Silu,
                                bias=bias_t[:, bidx : bidx + 1],
                            )
                        else:
                            sil2 = sil2p.tile(
                                [P, CH], BF16, tag="sil2",
                                name=f"sil2_{c0}_{m}",
                            )
                            nc.scalar.activation(
                                sil2[:],
                                ps[i][:],
                                mybir.ActivationFunctionType.Silu,
                                bias=bias_t[:, bidx : bidx + 1],
                            )
                            outs_t = outp.tile(
                                [P, CH], BF16, tag="outs",
                                name=f"outs_{c0}_{m}",
                            )
                            nc.vector.tensor_add(
                                outs_t[:], sil1_t[:, m, c0:c1], sil2[:]
                            )
                            nc.sync.dma_start(
                                outs_d[m * P : (m + 1) * P, c0:c1], outs_t[:]
                            )

            # --- routed expert, m-outer k-inner (mid-kernel, DMA caught up):
            # per (m,k) the column chunks share one weight load; the last
            # m-tile splits its tail chunk so the final drain is short.
            tag_ctr = 0
            for m in range(KT):
                if m < KT - 1:
                    r_chunks = [(0, CH), (CH, S)]
                else:
                    # last m-tile: c1 first (its silu overlaps c0's final
                    # matmuls), then c0 in 256-col halves whose stores go on
                    # different HWDGE rings, so the end-of-kernel silu/store/
                    # receipt chains overlap as much as possible
                    r_chunks = [(CH, S), (0, 256), (256, CH)]
                psums = []
                for ci, (c0, c1) in enumerate(r_chunks):
                    psums.append(
                        psump.tile(
                            [P, c1 - c0], F32, tag=f"ps{tag_ctr % 8}",
                            name=f"psr_{m}_{ci}",
                        )
                    )
                    tag_ctr += 1
                for k in range(KT):
                    lhsT = w_t[:, 2, k, m * P : (m + 1) * P]
                    for ci, (c0, c1) in enumerate(r_chunks):
                        nc.tensor.matmul(
                            psums[ci][:],
                            lhsT,
                            xu_t[:, k, c0:c1],
                            start=(k == 0),
                            stop=(k == KT - 1),
                        )
                bidx = 2 * KT + m
                for ci, (c0, c1) in enumerate(r_chunks):
                    outr_t = outp.tile(
                        [P, c1 - c0], BF16, tag="outr", name=f"outr_{m}_{ci}"
                    )
                    nc.scalar.activation(
                        outr_t[:],
                        psums[ci][:],
                        mybir.ActivationFunctionType.Silu,
                        bias=bias_t[:, bidx : bidx + 1],
                    )
                    eng = nc.scalar if (m == KT - 1 and ci == 2) else nc.sync
                    eng.dma_start(
                        outr_d[m * P : (m + 1) * P, c0:c1], outr_t[:]
                    )

    nc.compile()
    return nc


def _get_program():
    if "prog" not in _program_cache:
        _program_cache["prog"] = _build_program()
    return _program_cache["prog"]


def _silu(z):
    return z / (1.0 + np.exp(-z))


def kernel(x, expert_idx, Wr, br, Ws, bs):
    global last_results, last_nc, last_in_maps

    x = np.asarray(x, dtype=np.float32)
    idx = np.asarray(expert_idx).astype(np.int64)
    Wr = np.asarray(Wr, dtype=np.float32)
    br = np.asarray(br, dtype=np.float32)
    Ws = np.asarray(Ws, dtype=np.float32)
    bs = np.asarray(bs, dtype=np.float32)

    n_tokens = x.shape[0]
    assert x.shape == (N_CORES * S, D), f"unexpected x shape {x.shape}"

    # --- host-side "all-to-all": group tokens by expert ---
    order = np.argsort(idx, kind="stable")
    counts = np.bincount(idx, minlength=N_EXPERTS)
    offsets = np.zeros(N_EXPERTS + 1, dtype=np.int64)
    np.cumsum(counts, out=offsets[1:])

    x_sorted = x[order]
    x_sorted_bf = x_sorted.astype(ml_dtypes.bfloat16)

    Wr_bf = Wr.astype(ml_dtypes.bfloat16)
    Ws_bf = Ws.astype(ml_dtypes.bfloat16)

    in_maps = []
    for e in range(N_CORES):
        # device layout [P, KT, S]: xu[p, k, u] = x_win[k*P + p, u], so every
        # DMA slice is a contiguous >=1KB run per partition
        xu = np.ascontiguousarray(
            x_sorted_bf[e * S : (e + 1) * S].T.reshape(KT, P, S).transpose(1, 0, 2)
        )

        W = np.empty((NJ, D, D), dtype=ml_dtypes.bfloat16)
        W[0] = Ws_bf[0]
        W[1] = Ws_bf[1]
        W[2] = Wr_bf[e]

        # b[p, j*KT + m] = bias_j[m*P + p]
        b = np.empty((P, NJ * KT), dtype=np.float32)
        for j, bias in enumerate((bs[0], bs[1], br[e])):
            b[:, j * KT : (j + 1) * KT] = bias.reshape(KT, P).T

        in_maps.append({"xu": xu, "W": W, "b": b})

    nc = _get_program()
    res = bass_utils.run_bass_kernel_spmd(nc, in_maps, core_ids=list(range(N_CORES)))
    last_results = res
    last_nc = nc
    last_in_maps = in_maps

    # combine in sorted-token space, then permute back to input order
    out_sorted = np.concatenate(
        [np.asarray(res.results[e]["outs"]).astype(np.float32).T for e in range(N_CORES)],
        axis=0,
    )
    for e in range(N_CORES):
        if counts[e] == 0:
            continue
        p = np.arange(offsets[e], offsets[e + 1])
        inside = (p >= e * S) & (p < (e + 1) * S)
        win = p[inside]
        if len(win):
            outr = np.asarray(res.results[e]["outr"]).astype(np.float32)
            out_sorted[win] += outr[:, win - e * S].T
        ex = p[~inside]
        if len(ex):
            # count-skew overflow tokens: routed expert computed host-side
            # in f32 as part of the gather (<=1% of tokens)
            out_sorted[ex] += _silu(x_sorted[ex] @ Wr[e] + br[e])
    out = np.empty_like(out_sorted)
    out[order] = out_sorted
    return out[:n_tokens]


# revision 25
# speedup vs baseline: 1.0075x; 1.0075x over previous
"""MoE expert-collection kernel for 8 Trainium2 NeuronCores.

Problem (hardcoded shapes):
  x          [8192, 1024] f32
  expert_idx [8192]       int    (values 0..7)
  Wr         [8, 1024, 1024] f32, br [8, 1024] f32   (routing experts)
  Ws         [2, 1024, 1024] f32, bs [2, 1024] f32   (shared experts)
  out[n] = silu(x[n] @ Wr[e_n] + br[e_n]) + sum_s silu(x[n] @ Ws[s] + bs[s])

Strategy (expert parallel, host-side all-to-all):
  - Host sorts tokens by expert. Core e computes, over the fixed sorted-order
    token window [e*1024, (e+1)*1024):
      * silu(x @ Ws[0] + bs[0]) + silu(x @ Ws[1] + bs[1])  (shared experts,
        data-parallel over tokens -> perfectly balanced)
      * silu(x @ Wr[e] + br[e])  (routing expert e; the window approximates
        expert e's token range, so almost every routed token is covered)
  - Host combines: out = concat(shared windows); window tokens of expert e
    get the routed rows added from core e's output. The <=1% of routed
    tokens that fall outside their expert's window (count skew) are computed
    on the host in f32 during the gather -- this keeps the device program a
    uniform SPMD shape with zero padding columns.
  - Matmuls run in bf16 (fp32 PSUM accumulation): 1 PE cycle/row vs 4 for f32.

Schedule (built from trace analysis):
  - Phase order: shared j=0 (Ws0) -> shared j=1 (Ws1) -> routed (Wr[e]).
  - Shared phases are k-OUTER over (m-group, 512-col chunk) tiles with all 8
    PSUM banks live (two sets of 4 m-tiles, alternating per phase so a
    bank's silu has a full ~7us phase to drain before reuse). The first
    phase's working set is only half a W k-tile + one xu k-tile chunk per
    k-step, so the PE starts on real data as soon as the first two ~128KB
    DMAs land (~10.5us, right after the fixed ~7us engine preamble).
  - A short 5-matmul warmup on a memset tile occupies the PE from ~7.3us to
    ~10.5us: it releases the HAM clock throttle (cold PE runs at 1.2GHz vs
    2.4GHz warm) ~3us earlier without delaying the first real matmul.
  - Weight loads go on the scalar HWDGE ring (m-group halves, interleaved
    per k to match phase consumption), activation loads + all stores on the
    sync ring. Stores only begin after all loads drained. xu is staged
    host-side in [P, KT, S] layout so every DMA is a contiguous >=1KB run
    per partition.
  - Outputs are stored as bf16 (host upcasts): halves store traffic so DMA
    never contends with the weight prefetch.
  - The routed phase is m-outer/k-inner (per (m,k) both column chunks share
    one weight load); the last m-tile splits its second chunk into 2x256 so
    the end-of-kernel silu->store drain is minimal.
"""

import contextlib
import ctypes
import sys
import types

import numpy as np
import ml_dtypes

import concourse.mybir as mybir
import concourse.tile as tile
from concourse import bacc
from concourse import bass_utils

N_CORES = 8
D = 1024          # d_in == d_out
P = 128           # partitions
KT = D // P       # 8 k-tiles
NJ = 3            # matrices per core: Ws[0], Ws[1], Wr[e]
N_EXPERTS = 8
S = 8192 // N_CORES  # tokens per core (1024)
CH = 512          # PSUM bank chunk (512 fp32)

BF16 = mybir.dt.bfloat16
F32 = mybir.dt.float32

# exposed for test.py introspection
last_results = None
last_nc = None
last_in_maps = None

_program_cache = {}


def _install_ntff_hook_fallback():
    """Some containers (including this one) lack antenv.axon_hooks, but
    concourse's run_bass_kernel_spmd imports it unconditionally when tracing
    is requested (BASS_TRACE=1). Provide it: a ctypes port of
    trn_boot._ntff_profile_via_ctypes driving NRT profiling through the axon
    PJRT plugin, or a None hook (= trace gracefully skipped) if unavailable."""
    if "antenv.axon_hooks" in sys.modules:
        return
    try:
        import antenv.axon_hooks  # noqa: F401
        return
    except ImportError:
        pass

    hook = None
    try:
        lib = ctypes.CDLL("/opt/axon/libaxon_pjrt.so")
        if hasattr(lib, "axon_start_nrt_profile"):
            lib.axon_start_nrt_profile.argtypes = [
                ctypes.POINTER(ctypes.c_int64),
                ctypes.c_size_t,
            ]
            lib.axon_start_nrt_profile.restype = ctypes.c_int64
            lib.axon_stop_nrt_profile.argtypes = [ctypes.c_char_p]
            lib.axon_stop_nrt_profile.restype = ctypes.c_int64

            @contextlib.contextmanager
            def _hook(output_dir, device_ids):
                import jax

                jax.devices()  # force PJRT init so the axon client exists
                if device_ids:
                    ids = (ctypes.c_int64 * len(device_ids))(*device_ids)
                    rc = lib.axon_start_nrt_profile(ids, len(device_ids))
                else:
                    rc = lib.axon_start_nrt_profile(None, 0)
                if rc != 0:
                    raise RuntimeError(f"axon_start_nrt_profile rc={rc}")
                try:
                    yield
                finally:
                    n = lib.axon_stop_nrt_profile(str(output_dir).encode())
                    if n < 0:
                        raise RuntimeError(f"axon_stop_nrt_profile rc={n}")

            hook = _hook
    except OSError:
        pass

    mod = types.ModuleType("antenv.axon_hooks")
    mod.get_axon_ntff_profile_hook = lambda: hook
    mod.set_axon_ntff_profile_hook = lambda h: None
    sys.modules["antenv.axon_hooks"] = mod


_install_ntff_hook_fallback()


def _build_program():
    nc = bacc.Bacc(
        "TRN2",
        target_bir_lowering=False,
        debug=False,
        enable_asserts=False,
        num_devices=N_CORES,
    )
    xu_d = nc.dram_tensor("xu", [P, KT, S], BF16, kind="ExternalInput")
    W_d = nc.dram_tensor("W", [NJ, D, D], BF16, kind="ExternalInput")
    b_d = nc.dram_tensor("b", [P, NJ * KT], F32, kind="ExternalInput")
    outr_d = nc.dram_tensor("outr", [D, S], BF16, kind="ExternalOutput")
    outs_d = nc.dram_tensor("outs", [D, S], BF16, kind="ExternalOutput")

    with tile.TileContext(nc) as tc:
        with (
            tc.tile_pool(name="const", bufs=1) as constp,
            tc.tile_pool(name="wpool", bufs=1) as wp,
            tc.tile_pool(name="xpool", bufs=1) as xp,
            tc.tile_pool(name="sil1p", bufs=1) as sil1p,
            tc.tile_pool(name="sil2p", bufs=3) as sil2p,
            tc.tile_pool(name="outp", bufs=4) as outp,
            tc.tile_pool(name="psum", bufs=1, space="PSUM") as psump,
        ):
            w_t = wp.tile([P, NJ, KT, D], BF16)
            xu_t = xp.tile([P, KT, S], BF16)
            bias_t = constp.tile([P, NJ * KT], F32)
            sil1_t = sil1p.tile([P, KT, S], BF16)

            # --- PE warmup: a memset tile + 5 dummy matmuls sized to end
            # right when the first real operands land (~10.5us), releasing
            # the HAM clock throttle early without delaying real work.
            warm_sb = constp.tile([P, 640], BF16, name="warm_sb")
            nc.vector.memset(warm_sb[:], 0.0)
            warm_ps = psump.tile([P, CH], F32, tag="ps0", name="warm_ps")
            for _ in range(5):
                nc.tensor.matmul(
                    warm_ps[:], warm_sb[:, :P], warm_sb[:, P : P + CH],
                    start=True, stop=True,
                )

            # --- input DMA: two parallel HWDGE issue streams, ordered by
            # first-use so the wires never carry "future" data during the
            # ramp. scalar ring: weights, split into m-group halves
            # (phase (mgA,c0) consumes h0 k-tiles, phase (mgB,c1) h1).
            for h in range(2):
                for k in range(KT):
                    nc.scalar.dma_start(
                        w_t[:, 0, k, h * CH : (h + 1) * CH],
                        W_d[0, k * P : (k + 1) * P, h * CH : (h + 1) * CH],
                    )
            nc.scalar.dma_start(
                w_t[:, 1, :, :], W_d[1].rearrange("(kt p) n -> p kt n", p=P)
            )
            nc.scalar.dma_start(
                w_t[:, 2, :, :], W_d[2].rearrange("(kt p) n -> p kt n", p=P)
            )
            # sync ring: activations, k-paced
            for k in range(KT):
                nc.sync.dma_start(xu_t[:, k, 0:CH], xu_d[:, k, 0:CH])
            nc.sync.dma_start(bias_t[:], b_d[:])
            for k in range(KT):
                nc.sync.dma_start(xu_t[:, k, CH:S], xu_d[:, k, CH:S])

            # --- shared experts, k-outer: phase = (j, m-group, col-chunk).
            # Per j the order (A,c0),(B,c0),(A,c1),(B,c1) alternates the PSUM
            # bank set every phase and needs at most 1MB of fresh input per
            # phase (phase 1: W h0 + xu c0 paced per-k; phase 2: W h1 only;
            # phase 3: xu c1 only; phase 4 fully cached).
            for j in range(2):
                for mg, (c0, c1) in ((0, (0, CH)), (1, (0, CH)),
                                     (0, (CH, S)), (1, (CH, S))):
                    ps = [
                        psump.tile(
                            [P, CH], F32, tag=f"ps{mg * 4 + i}",
                            name=f"pss_{j}_{mg}_{c0}_{i}",
                        )
                        for i in range(4)
                    ]
                    for k in range(KT):
                        for i in range(4):
                            m = mg * 4 + i
                            nc.tensor.matmul(
                                ps[i][:],
                                w_t[:, j, k, m * P : (m + 1) * P],
                                xu_t[:, k, c0:c1],
                                start=(k == 0),
                                stop=(k == KT - 1),
                            )
                    for i in range(4):
                        m = mg * 4 + i
                        bidx = j * KT + m
                        if j == 0:
                            nc.scalar.activation(
                                sil1_t[:, m, c0:c1],
                                ps[i][:],
                                mybir.ActivationFunctionType.Silu,
                                bias=bias_t[:, bidx : bidx + 1],
                            )
                        else:
                            sil2 = sil2p.tile(
                                [P, CH], BF16, tag="sil2",
                                name=f"sil2_{c0}_{m}",
                            )
                            nc.scalar.activation(
                                sil2[:],
                                ps[i][:],
                                mybir.ActivationFunctionType.Silu,
                                bias=bias_t[:, bidx : bidx + 1],
                            )
                            outs_t = outp.tile(
                                [P, CH], BF16, tag="outs",
                                name=f"outs_{c0}_{m}",
                            )
                            nc.vector.tensor_add(
                                outs_t[:], sil1_t[:, m, c0:c1], sil2[:]
                            )
                            nc.sync.dma_start(
                                outs_d[m * P : (m + 1) * P, c0:c1], outs_t[:]
                            )

            # --- routed expert, m-outer k-inner (mid-kernel, DMA caught up):
            # per (m,k) the column chunks share one weight load; the last
            # m-tile splits its tail chunk so the final drain is short.
            tag_ctr = 0
            for m in range(KT):
                if m < KT - 1:
                    r_chunks = [(0, CH), (CH, S)]
                else:
                    # last m-tile: c1 first (its silu overlaps c0's final
                    # matmuls), then c0 in 256-col halves whose stores go on
                    # different HWDGE rings, so the end-of-kernel silu/store/
                    # receipt chains overlap as much as possible
                    r_chunks = [(CH, S), (0, 256), (256, CH)]
                psums = []
                for ci, (c0, c1) in enumerate(r_chunks):
                    psums.append(
                        psump.tile(
                            [P, c1 - c0], F32, tag=f"ps{tag_ctr % 8}",
                            name=f"psr_{m}_{ci}",
                        )
                    )
                    tag_ctr += 1
                for k in range(KT):
                    lhsT = w_t[:, 2, k, m * P : (m + 1) * P]
                    for ci, (c0, c1) in enumerate(r_chunks):
                        nc.tensor.matmul(
                            psums[ci][:],
                            lhsT,
                            xu_t[:, k, c0:c1],
                            start=(k == 0),
                            stop=(k == KT - 1),
                        )
                bidx = 2 * KT + m
                for ci, (c0, c1) in enumerate(r_chunks):
                    outr_t = outp.tile(
                        [P, c1 - c0], BF16, tag="outr", name=f"outr_{m}_{ci}"
                    )
                    nc.scalar.activation(
                        outr_t[:],
                        psums[ci][:],
                        mybir.ActivationFunctionType.Silu,
                        bias=bias_t[:, bidx : bidx + 1],
                    )
                    eng = nc.scalar if (m == KT - 1 and ci == 2) else nc.sync
                    eng.dma_start(
                        outr_d[m * P : (m + 1) * P, c0:c1], outr_t[:]
                    )

    nc.compile()
    return nc


def _get_program():
    if "prog" not in _program_cache:
        _program_cache["prog"] = _build_program()
    return _program_cache["prog"]


def _silu(z):
    return z / (1.0 + np.exp(-z))


def kernel(x, expert_idx, Wr, br, Ws, bs):
    global last_results, last_nc, last_in_maps

    x = np.asarray(x, dtype=np.float32)
    idx = np.asarray(expert_idx).astype(np.int64)
    Wr = np.asarray(Wr, dtype=np.float32)
    br = np.asarray(br, dtype=np.float32)
    Ws = np.asarray(Ws, dtype=np.float32)
    bs = np.asarray(bs, dtype=np.float32)

    n_tokens = x.shape[0]
    assert x.shape == (N_CORES * S, D), f"unexpected x shape {x.shape}"

    # --- host-side "all-to-all": group tokens by expert ---
    order = np.argsort(idx, kind="stable")
    counts = np.bincount(idx, minlength=N_EXPERTS)
    offsets = np.zeros(N_EXPERTS + 1, dtype=np.int64)
    np.cumsum(counts, out=offsets[1:])

    x_sorted = x[order]
    x_sorted_bf = x_sorted.astype(ml_dtypes.bfloat16)

    Wr_bf = Wr.astype(ml_dtypes.bfloat16)
    Ws_bf = Ws.astype(ml_dtypes.bfloat16)

    in_maps = []
    for e in range(N_CORES):
        # device layout [P, KT, S]: xu[p, k, u] = x_win[k*P + p, u], so every
        # DMA slice is a contiguous >=1KB run per partition
        xu = np.ascontiguousarray(
            x_sorted_bf[e * S : (e + 1) * S].T.reshape(KT, P, S).transpose(1, 0, 2)
        )

        W = np.empty((NJ, D, D), dtype=ml_dtypes.bfloat16)
        W[0] = Ws_bf[0]
        W[1] = Ws_bf[1]
        W[2] = Wr_bf[e]

        # b[p, j*KT + m] = bias_j[m*P + p]
        b = np.empty((P, NJ * KT), dtype=np.float32)
        for j, bias in enumerate((bs[0], bs[1], br[e])):
            b[:, j * KT : (j + 1) * KT] = bias.reshape(KT, P).T

        in_maps.append({"xu": xu, "W": W, "b": b})

    nc = _get_program()
    res = bass_utils.run_bass_kernel_spmd(nc, in_maps, core_ids=list(range(N_CORES)))
    last_results = res
    last_nc = nc
    last_in_maps = in_maps

    # combine in sorted-token space, then permute back to input order
    out_sorted = np.concatenate(
        [np.asarray(res.results[e]["outs"]).astype(np.float32).T for e in range(N_CORES)],
        axis=0,
    )
    for e in range(N_CORES):
        if counts[e] == 0:
            continue
        p = np.arange(offsets[e], offsets[e + 1])
        inside = (p >= e * S) & (p < (e + 1) * S)
        win = p[inside]
        if len(win):
            outr = np.asarray(res.results[e]["outr"]).astype(np.float32)
            out_sorted[win] += outr[:, win - e * S].T
        ex = p[~inside]
        if len(ex):
            # count-skew overflow tokens: routed expert computed host-side
            # in f32 as part of the gather (<=1% of tokens)
            out_sorted[ex] += _silu(x_sorted[ex] @ Wr[e] + br[e])
    out = np.empty_like(out_sorted)
    out[order] = out_sorted
    return out[:n_tokens]
